# revision 23
# baseline (speedup 1.0000x reference)
"""DeepAR (2-layer LSTM, H=512) Trainium2 Bass kernel, 8-core data-parallel.

Model (see reference): x = concat(x_cont, emb0[cat0], emb1[cat1]) [B,T,56]
  -> LSTM(512) -> LSTM(512) -> mu = h@Wmu+bmu ; sigma = softplus(h@Wsig+bsig)

Sharding: batch B=256 split across 8 cores (32 rows each); params replicated.

Per-core device program — transposed-gates formulation. All recurrent
matmuls put the WEIGHTS in the stationary operand and stream h^T, so each
matmul's moving dim is just the 32-row batch:

  gates^T [2048, 32] = sum_c Wr[c]^T-tiles @ h^T_c  (+ [Wk;b] @ [x^T;1])

The PE streams 16 Mtiles x (1 xz + 4 Wr) x 32 rows for L1 and
16 x (4 Wk2 + 4 Wr2) x 32 (+512-row b2 inject) for L2 — ~7.2K rows/step
vs ~21.5K for the batch-major formulation, and h^T is produced directly
by the elementwise tail (no per-step PE transposes).

  - gate tile order [i | f | o | g]: one fused Sigmoid over 12 tiles +
    one Tanh over 4; cell update and h = o*tanh(c) on DVE in bf16
  - both layers interleaved in one scan (L2 runs 1 step behind L1);
    PE order per macro step: Wr1[t], b2/Wk2/Wr2[t-1], head[t-1],
    xzb1[t+1] — the L1 ACT/DVE tail hides under the L2 matmuls
  - head: lhsT = h2^T chunk (stationary), rhs = Wms [128, 2] — 8 rows
    per step, accumulated 16 steps per PSUM bank, staged batch-major
  - embeddings gathered up-front on the Pool queue (96 single-index
    indirect DMAs; multi-index indirect DMA corrupts SBUF on HW), with
    the x^T PE transposes pipelined 2 tiles ahead of the scan
  - all Exp/Ln (softplus) deferred to one epilogue pass -> no ACT
    table swaps inside the scan
"""

import numpy as np
import ml_dtypes

import concourse.bass as bass
import concourse.mybir as mybir
import concourse.tile as tile
from concourse import bacc
from concourse.masks import make_identity

F32 = mybir.dt.float32
BF16 = mybir.dt.bfloat16
I32 = mybir.dt.int32

B, T, F = 256, 192, 8
CARD0, CARD1 = 1000, 100
E0, E1 = 32, 16
H = 512
DIN = F + E0 + E1          # 56
G4 = 4 * H                 # 2048
NC_N = 8                   # cores
BSH = B // NC_N            # 32 batch rows per core
R = T * BSH                # 6144 (t,b)-ordered rows per core
KC = H // 128              # 4 recurrent K-chunks
NM = G4 // 128             # 16 gate-column Mtiles
MT = R // 128              # 48 x^T column tiles
R1 = 4                     # h1^T ring depth
R2 = 8                     # h2^T ring depth
A = mybir.ActivationFunctionType

# gate slot order [i | f | o | g] (i: slots 0-3, f: 4-7, o: 8-11, g: 12-15)
_GATE_BASE = [0, H, 3 * H, 2 * H]   # orig col base per slot-group, z=[i|f|g|o]


def _colperm():
    """P[m*128+p] = original G4 column of (Mtile m, partition p)."""
    P = np.empty(G4, np.int64)
    for m in range(NM):
        base = _GATE_BASE[m // 4]
        chunk = m % 4
        P[m * 128:(m + 1) * 128] = base + chunk * 128 + np.arange(128)
    return P


_NC_CACHE = {}


def build_nc(upto="all"):
    if upto in _NC_CACHE:
        return _NC_CACHE[upto]
    nc = bacc.Bacc("TRN2", num_devices=NC_N)

    # ---------------- DRAM I/O ----------------
    idx0_d = nc.dram_tensor("idx0", [128, MT], I32, kind="ExternalInput")
    idx1_d = nc.dram_tensor("idx1", [128, MT], I32, kind="ExternalInput")
    e0t_d = nc.dram_tensor("e0tab", [CARD0, E0], F32, kind="ExternalInput")
    e1t_d = nc.dram_tensor("e1tab", [CARD1, E1], F32, kind="ExternalInput")
    xcr_d = nc.dram_tensor("xcr", [128, MT, F], F32, kind="ExternalInput")
    w1e_d = nc.dram_tensor("w1e", [64, NM, 128], BF16, kind="ExternalInput")
    wr1_d = nc.dram_tensor("wr1", [128, KC, NM, 128], BF16, kind="ExternalInput")
    wk2_d = nc.dram_tensor("wk2", [128, KC, NM, 128], BF16, kind="ExternalInput")
    wr2_d = nc.dram_tensor("wr2", [128, KC, NM, 128], BF16, kind="ExternalInput")
    b2m_d = nc.dram_tensor("b2m", [16, 128], BF16, kind="ExternalInput")
    wms_d = nc.dram_tensor("wms", [128, KC, 2], BF16, kind="ExternalInput")
    bms_d = nc.dram_tensor("bms", [1, 2], F32, kind="ExternalInput")
    dl16_d = nc.dram_tensor("dl16", [16, NM * BSH], BF16, kind="ExternalInput")

    mu_d = nc.dram_tensor("mu", [BSH, T], F32, kind="ExternalOutput")
    sg_d = nc.dram_tensor("sigma", [BSH, T], F32, kind="ExternalOutput")
    dbg_d = nc.dram_tensor("dbg", [128, T, BSH], F32, kind="ExternalOutput") \
        if upto != "all" else None

    _build_body(nc, upto, locals())
    nc.compile()
    _NC_CACHE[upto] = nc
    return nc


def _build_body(nc, upto, env):
    from contextlib import ExitStack
    idx0_d = env["idx0_d"]; idx1_d = env["idx1_d"]; xcr_d = env["xcr_d"]
    e0t_d = env["e0t_d"]; e1t_d = env["e1t_d"]; w1e_d = env["w1e_d"]
    wr1_d = env["wr1_d"]; wk2_d = env["wk2_d"]; wr2_d = env["wr2_d"]
    b2m_d = env["b2m_d"]; wms_d = env["wms_d"]; bms_d = env["bms_d"]
    dl16_d = env["dl16_d"]
    mu_d = env["mu_d"]; sg_d = env["sg_d"]; dbg_d = env["dbg_d"]

    with tile.TileContext(nc) as tc, ExitStack() as top:
        singles = top.enter_context(tc.tile_pool(name="singles", bufs=1))

        # ---------------- weights / constants to SBUF ----------------
        # gather inputs (idx/xcb) are DMA'd first — see phase 1 — so the
        # HWDGE queue isn't clogged by the 52KB/partition of weights
        w1e = singles.tile([64, NM, 128], BF16)
        wr1 = singles.tile([128, KC, NM, 128], BF16)
        wk2 = singles.tile([128, KC, NM, 128], BF16)
        wr2 = singles.tile([128, KC, NM, 128], BF16)
        b2m = singles.tile([16, 128], BF16)
        nc.sync.dma_start(out=b2m[:], in_=b2m_d[:])
        wms = singles.tile([128, KC, 2], BF16)
        nc.sync.dma_start(out=wms[:], in_=wms_d[:])
        bms = singles.tile([1, 2], F32)
        nc.sync.dma_start(out=bms[:], in_=bms_d[:])
        # bmu/bsig broadcast to 32 partitions for the epilogue
        bmu32 = singles.tile([BSH, 1], F32)
        nc.sync.dma_start(
            out=bmu32[:],
            in_=bass.AP(tensor=bms_d[:].tensor, offset=0, ap=[[0, BSH], [1, 1]]))
        bsg32 = singles.tile([BSH, 1], F32)
        nc.sync.dma_start(
            out=bsg32[:],
            in_=bass.AP(tensor=bms_d[:].tensor, offset=1, ap=[[0, BSH], [1, 1]]))

        ident_f32 = singles.tile([128, 128], F32)
        make_identity(nc, ident_f32[:])

        # delta16[k, (m, j)] = 1 if k == m else 0 — b2-inject rhs
        delta16 = singles.tile([16, NM, BSH], BF16)
        nc.sync.dma_start(out=delta16[:], in_=dl16_d[:])

        # h^T rings, cell states (bf16), head staging
        h1h = singles.tile([128, R1, KC, BSH], BF16)
        h2h = singles.tile([128, R2, KC, BSH], BF16)
        c1 = singles.tile([128, KC, BSH], BF16)
        c2 = singles.tile([128, KC, BSH], BF16)
        stage = singles.tile([BSH, T, 2], F32)

        # ---------------- phase 1: gather + x^T build ----------------
        gp = top.enter_context(tc.tile_pool(name="gather", bufs=1))
        idx0_sb = gp.tile([128, MT], I32)
        nc.sync.dma_start(out=idx0_sb[:], in_=idx0_d[:])
        idx1_sb = gp.tile([128, MT], I32)
        nc.sync.dma_start(out=idx1_sb[:], in_=idx1_d[:])
        # assembled rows: [p, m, 64] = [e0 | e1 | xc | ones(+pad)]
        asm = gp.tile([128, MT, 64], F32)
        nc.vector.memset(asm[:], 1.0)
        xcb = gp.tile([128, MT, F], F32)
        nc.sync.dma_start(out=xcb[:], in_=xcr_d[:])
        nc.vector.tensor_copy(asm[:, :, E0 + E1:DIN], xcb[:])
        # weights after the gather inputs; first-use order
        nc.sync.dma_start(out=w1e[:], in_=w1e_d[:])
        nc.sync.dma_start(out=wr1[:], in_=wr1_d[:])
        nc.sync.dma_start(out=wk2[:], in_=wk2_d[:])
        nc.sync.dma_start(out=wr2[:], in_=wr2_d[:])
        for m in range(MT):
            nc.gpsimd.indirect_dma_start(
                out=asm[:, m, 0:E0], out_offset=None, in_=e0t_d[:],
                in_offset=bass.IndirectOffsetOnAxis(
                    ap=idx0_sb[:, m:m + 1], axis=0))
            nc.gpsimd.indirect_dma_start(
                out=asm[:, m, E0:E0 + E1], out_offset=None, in_=e1t_d[:],
                in_offset=bass.IndirectOffsetOnAxis(
                    ap=idx1_sb[:, m:m + 1], axis=0))

        xT = top.enter_context(tc.tile_pool(name="xtp", bufs=1)).tile([64, R], BF16)
        ptr = top.enter_context(tc.tile_pool(name="gtr", bufs=2, space="PSUM"))

        def xT_tile(m):
            ps = ptr.tile([64, 128], F32, tag="tr")
            nc.tensor.transpose(ps[:], asm[:, m, :], ident_f32[:])
            nc.vector.tensor_copy(xT[:, 128 * m:128 * (m + 1)], ps[:])

        xT_tile(0)
        xT_tile(1)
        xT_done = 2

        if upto == "xT":
            for m in range(2, MT):
                xT_tile(m)
            with tc.tile_pool(name="dbgp", bufs=1) as dp:
                dbg_sb = dp.tile([64, R], F32)
                nc.vector.tensor_copy(dbg_sb[:], xT[:])
                dv = bass.AP(tensor=dbg_d[:].tensor, offset=0,
                             ap=[[T * BSH, 64], [1, R]])
                nc.sync.dma_start(out=dv, in_=dbg_sb[:])
            return
        dbg_sb = None
        if upto in ("h1", "h2"):
            dbg_sb = singles.tile([128, T, BSH], F32)

        # ---------------- main interleaved scan ----------------
        ew = top.enter_context(tc.tile_pool(name="ew", bufs=2))
        psg1 = top.enter_context(tc.tile_pool(name="psg1", bufs=2, space="PSUM"))
        psg2 = top.enter_context(tc.tile_pool(name="psg2", bufs=2, space="PSUM"))
        psh = top.enter_context(tc.tile_pool(name="psh", bufs=2, space="PSUM"))

        pg1 = {}
        pg2 = {}
        psH = [None]

        def b2inj(s):
            # starts the L2 psum group for step s (zeroes the bank)
            pg = psg2.tile([128, NM, BSH], F32, tag="pg2", name="pg2")
            pg2[s] = pg
            nc.tensor.matmul(pg[:], b2m[:], delta16[:],
                             start=True, stop=False, skip_group_check=True)

        def xzb1(t):
            # input projection + b1 for step t (starts the psum group)
            pg = psg1.tile([128, NM, BSH], F32, tag="pg1")
            pg1[t] = pg
            # start=True zeroes the whole 2KB zero region (= this bank):
            # only the first matmul into the bank may set it
            for m in range(NM):
                nc.tensor.matmul(pg[:, m, :], w1e[0:57, m, :],
                                 xT[0:57, t * BSH:(t + 1) * BSH],
                                 start=(m == 0),
                                 stop=(t == 0 and m == NM - 1),
                                 skip_group_check=True)

        # emit o-gate tiles first so sigmoid(o) overlaps the i/f/g matmuls;
        # g last (tanh(g) is the next chain dependency after sigmoid(i,f))
        M_ORDER = [8, 9, 10, 11, 0, 1, 2, 3, 4, 5, 6, 7, 12, 13, 14, 15]

        def wr_mm(pg, w, hring, rslot, t0):
            for m in M_ORDER:
                for c in range(KC):
                    nc.tensor.matmul(pg[:, m, :], w[:, c, m, :],
                                     hring[:, rslot, c, :],
                                     start=False,
                                     stop=(t0 and m == M_ORDER[-1]
                                           and c == KC - 1),
                                     skip_group_check=True)

        def tail(t, pg, cst, hring, rslot):
            # gates^T [128, 16, 32] -> h^T ring slot; slots [i | f | o | g]
            sgo = ew.tile([128, KC, BSH], BF16, tag="sgo")
            nc.scalar.activation(sgo[:], pg[:, 8:12, :], A.Sigmoid)
            sif = ew.tile([128, 2 * KC, BSH], BF16, tag="sif")
            nc.scalar.activation(sif[:], pg[:, 0:8, :], A.Sigmoid)
            tng = ew.tile([128, KC, BSH], BF16, tag="tng")
            nc.scalar.activation(tng[:], pg[:, 12:16, :], A.Tanh)
            ig = ew.tile([128, KC, BSH], BF16, tag="ig")
            if t > 0:
                fc = ew.tile([128, KC, BSH], BF16, tag="fc")
                nc.vector.tensor_mul(fc[:], sif[:, KC:2 * KC, :], cst[:])
                nc.vector.tensor_mul(ig[:], sif[:, 0:KC, :], tng[:])
                nc.vector.tensor_add(cst[:], fc[:], ig[:])
            else:
                nc.vector.tensor_mul(ig[:], sif[:, 0:KC, :], tng[:])
                nc.vector.tensor_copy(cst[:], ig[:])
            tnc = ew.tile([128, KC, BSH], BF16, tag="tnc")
            nc.scalar.activation(tnc[:], cst[:], A.Tanh)
            nc.vector.tensor_mul(hring[:, rslot, :, :], sgo[:], tnc[:])

        def head(s):
            # mu/sigma for step s: out [32, 2] <- sum_c h2^T_c(stat) @ wms_c
            if s % 16 == 0:
                psH[0] = psh.tile([BSH, 16, 2], F32, tag="psH", name="psH")
            for c in range(KC):
                nc.tensor.matmul(psH[0][:, s % 16, :], h2h[:, s % R2, c, :],
                                 wms[:, c, :],
                                 start=(c == 0 and s % 16 == 0),
                                 stop=(c == KC - 1 and s % 16 == 15),
                                 skip_group_check=True)
            if s % 16 == 15:
                nc.vector.tensor_copy(stage[:, s - 15:s + 1, :], psH[0][:])

        # macro step t: Wr1[t] -> L1 tail[t] -> L2[t-1] -> head[t-2]
        # -> xzb1[t+1], b2[t] (+ pipelined x^T transposes); the head lags
        # one more macro so it never waits on the L2 tail just emitted
        xzb1(0)
        for t in range(T + 2):
            if t < T:
                if t > 0:
                    wr_mm(pg1[t], wr1, h1h, (t - 1) % R1, True)
                tail(t, pg1.pop(t), c1, h1h, t % R1)
            s = t - 1
            if 0 <= s < T:
                b2inj(s)
                pg = pg2.pop(s)
                wr_mm(pg, wk2, h1h, s % R1, s == 0)
                if s > 0:
                    wr_mm(pg, wr2, h2h, (s - 1) % R2, True)
                tail(s, pg, c2, h2h, s % R2)
            if 0 <= t - 2 < T:
                head(t - 2)
            if t + 1 < T:
                xzb1(t + 1)
            while xT_done * 4 < t + 10 and xT_done < MT:
                xT_tile(xT_done)
                xT_done += 1
            if upto == "h1" and t < T:
                nc.vector.tensor_copy(dbg_sb[:, t, :], h1h[:, t % R1, 0, :])
            if upto == "h2" and 0 <= s < T:
                nc.vector.tensor_copy(dbg_sb[:, s, :], h2h[:, s % R2, 0, :])

        # ---------------- epilogue: bias, softplus, DMA out ----------------
        ep = top.enter_context(tc.tile_pool(name="ep", bufs=1))
        mu_sb = ep.tile([BSH, T], F32)
        nc.vector.tensor_scalar_add(mu_sb[:], stage[:, :, 0], bmu32[:])
        sg_e = ep.tile([BSH, T], F32)
        nc.scalar.activation(sg_e[:], stage[:, :, 1], A.Exp, bias=bsg32[:])
        sg_sb = ep.tile([BSH, T], F32)
        nc.scalar.activation(sg_sb[:], sg_e[:], A.Ln, bias=1.0)
        nc.sync.dma_start(out=mu_d[:], in_=mu_sb[:])
        nc.sync.dma_start(out=sg_d[:], in_=sg_sb[:])
        if upto in ("h1", "h2"):
            nc.sync.dma_start(out=dbg_d[:], in_=dbg_sb[:])

    return nc


def _marshal(inputs):
    """Host-side shard/layout marshalling (no compute beyond dtype cast/pad)."""
    bf = ml_dtypes.bfloat16
    xc = np.ascontiguousarray(np.asarray(inputs["x_cont"], np.float32))
    cat0 = np.asarray(inputs["cat0"]).astype(np.int32)
    cat1 = np.asarray(inputs["cat1"]).astype(np.int32)
    emb0 = np.asarray(inputs["emb0"], np.float32)
    emb1 = np.asarray(inputs["emb1"], np.float32)
    Wk1 = np.asarray(inputs["Wk1"], np.float32)
    Wr1 = np.asarray(inputs["Wr1"], np.float32)
    b1 = np.asarray(inputs["b1"], np.float32)
    Wk2 = np.asarray(inputs["Wk2"], np.float32)
    Wr2 = np.asarray(inputs["Wr2"], np.float32)
    b2 = np.asarray(inputs["b2"], np.float32)
    Wmu = np.asarray(inputs["Wmu"], np.float32)
    bmu = np.asarray(inputs["bmu"], np.float32)
    Wsig = np.asarray(inputs["Wsig"], np.float32)
    bsig = np.asarray(inputs["bsig"], np.float32)

    P = _colperm()

    # xT partition order: 0-31 emb0 dims, 32-47 emb1 dims, 48-55 x_cont, 56 ones
    wk1_rows = np.concatenate([Wk1[F:F + E0], Wk1[F + E0:DIN], Wk1[0:F],
                               b1.reshape(1, G4)], axis=0)      # [57, G4]
    w1e = np.zeros((64, NM, 128), bf)
    w1e[0:57] = wk1_rows[:, P].reshape(57, NM, 128).astype(bf)

    def kperm(W):  # [512, G4] -> [128(k), KC, NM, 128]
        Wp = W[:, P].reshape(KC, 128, NM, 128)
        return np.ascontiguousarray(Wp.transpose(1, 0, 2, 3)).astype(bf)

    wr1 = kperm(Wr1)
    wk2 = kperm(Wk2)
    wr2 = kperm(Wr2)
    b2m = np.ascontiguousarray(b2[P].reshape(16, 128)).astype(bf)
    wms = np.zeros((128, KC, 2), bf)
    for c in range(KC):
        wms[:, c, 0] = Wmu[c * 128:(c + 1) * 128, 0].astype(bf)
        wms[:, c, 1] = Wsig[c * 128:(c + 1) * 128, 0].astype(bf)
    bms = np.array([[float(bmu.reshape(-1)[0]), float(bsig.reshape(-1)[0])]],
                   np.float32)
    dl16 = np.kron(np.eye(16, dtype=np.float32),
                   np.ones((1, BSH), np.float32)).astype(bf)  # [16, 512]

    def wrap_idx(cat):  # [BSH, T] -> (t,b) rows -> [128, MT] int32
        lin = np.ascontiguousarray(cat.T).reshape(-1)       # (t, b) order
        return np.ascontiguousarray(lin.reshape(MT, 128).T.astype(np.int32))

    in_maps = []
    for cidx in range(NC_N):
        sl = slice(cidx * BSH, (cidx + 1) * BSH)
        xcs = xc[sl]                                        # [32, 192, 8]
        rows = xcs.transpose(1, 0, 2).reshape(R, F)         # (t,b) rows
        xcr = np.ascontiguousarray(
            rows.reshape(MT, 128, F).transpose(1, 0, 2).astype(np.float32))
        in_maps.append({
            "xcr": xcr,
            "idx0": wrap_idx(cat0[sl]),
            "idx1": wrap_idx(cat1[sl]),
            "e0tab": emb0, "e1tab": emb1,
            "w1e": w1e, "wr1": wr1, "wk2": wk2, "wr2": wr2,
            "b2m": b2m, "wms": wms, "bms": bms, "dl16": dl16,
        })
    return in_maps


_RUN_KWARGS = {}   # test harness may set e.g. {"trace": True} for profiling
_LAST_RESULT = []


def kernel(**inputs):
    from concourse.bass_utils import run_bass_kernel_spmd
    in_maps = _marshal(inputs)
    nc = build_nc()
    res = run_bass_kernel_spmd(nc, in_maps, core_ids=list(range(NC_N)),
                               **_RUN_KWARGS)
    _LAST_RESULT.clear()
    _LAST_RESULT.append(res)
    mu = np.concatenate([r["mu"] for r in res.results], axis=0)      # [256, 192]
    sg = np.concatenate([r["sigma"] for r in res.results], axis=0)
    return (mu.reshape(B, T, 1).astype(np.float32),
            sg.reshape(B, T, 1).astype(np.float32))


# revision 30
# speedup vs baseline: 1.2061x; 1.2061x over previous
"""DeepAR (2-layer LSTM, H=512) Trainium2 Bass kernel, 8-core data-parallel.

Model (see reference): x = concat(x_cont, emb0[cat0], emb1[cat1]) [B,T,56]
  -> LSTM(512) -> LSTM(512) -> mu = h@Wmu+bmu ; sigma = softplus(h@Wsig+bsig)

Sharding: batch B=256 split across 8 cores (32 rows each); params replicated.

Per-core device program — transposed-gates formulation. All recurrent
matmuls put the WEIGHTS in the stationary operand and stream h^T, so each
matmul's moving dim is just the 32-row batch:

  gates^T [2048, 32] = sum_c Wr[c]^T-tiles @ h^T_c  (+ [Wk;b] @ [x^T;1])

The PE streams 16 Mtiles x (1 xz + 4 Wr) x 32 rows for L1 and
16 x (4 Wk2 + 4 Wr2) x 32 (+512-row b2 inject) for L2 — ~7.2K rows/step
vs ~21.5K for the batch-major formulation, and h^T is produced directly
by the elementwise tail (no per-step PE transposes).

  - gate tile order [i | f | o | g]: one fused Sigmoid over 12 tiles +
    one Tanh over 4; cell update and h = o*tanh(c) on DVE in bf16
  - both layers interleaved in one scan (L2 runs 1 step behind L1);
    PE order per macro step: Wr1[t], b2/Wk2/Wr2[t-1], head[t-1],
    xzb1[t+1] — the L1 ACT/DVE tail hides under the L2 matmuls
  - head: lhsT = h2^T chunk (stationary), rhs = Wms [128, 2] — 8 rows
    per step, accumulated 16 steps per PSUM bank, staged batch-major
  - embeddings gathered up-front on the Pool queue (96 single-index
    indirect DMAs; multi-index indirect DMA corrupts SBUF on HW), with
    the x^T PE transposes pipelined 2 tiles ahead of the scan
  - all Exp/Ln (softplus) deferred to one epilogue pass -> no ACT
    table swaps inside the scan
"""

import numpy as np
import ml_dtypes

import concourse.bass as bass
import concourse.mybir as mybir
import concourse.tile as tile
from concourse import bacc
from concourse.masks import make_identity

F32 = mybir.dt.float32
BF16 = mybir.dt.bfloat16
I32 = mybir.dt.int32

B, T, F = 256, 192, 8
CARD0, CARD1 = 1000, 100
E0, E1 = 32, 16
H = 512
DIN = F + E0 + E1          # 56
G4 = 4 * H                 # 2048
NC_N = 8                   # cores
BSH = B // NC_N            # 32 batch rows per core
R = T * BSH                # 6144 (t,b)-ordered rows per core
KC = H // 128              # 4 recurrent K-chunks
NM = G4 // 128             # 16 gate-column Mtiles
MT = R // 128              # 48 x^T column tiles
R1 = 4                     # h1^T ring depth
R2 = 8                     # h2^T ring depth
A = mybir.ActivationFunctionType

# gate slot order [i | f | o | g] (i: slots 0-3, f: 4-7, o: 8-11, g: 12-15)
_GATE_BASE = [0, H, 3 * H, 2 * H]   # orig col base per slot-group, z=[i|f|g|o]


def _colperm():
    """P[m*128+p] = original G4 column of (Mtile m, partition p)."""
    P = np.empty(G4, np.int64)
    for m in range(NM):
        base = _GATE_BASE[m // 4]
        chunk = m % 4
        P[m * 128:(m + 1) * 128] = base + chunk * 128 + np.arange(128)
    return P


_NC_CACHE = {}


def build_nc(upto="all"):
    if upto in _NC_CACHE:
        return _NC_CACHE[upto]
    nc = bacc.Bacc("TRN2", num_devices=NC_N)

    # ---------------- DRAM I/O ----------------
    idx0_d = nc.dram_tensor("idx0", [128, MT], I32, kind="ExternalInput")
    idx1_d = nc.dram_tensor("idx1", [128, MT], I32, kind="ExternalInput")
    e0t_d = nc.dram_tensor("e0tab", [CARD0, E0], F32, kind="ExternalInput")
    e1t_d = nc.dram_tensor("e1tab", [CARD1, E1], F32, kind="ExternalInput")
    xcr_d = nc.dram_tensor("xcr", [128, MT, F], F32, kind="ExternalInput")
    w1e_d = nc.dram_tensor("w1e", [64, NM, 128], BF16, kind="ExternalInput")
    wr1_d = nc.dram_tensor("wr1", [128, KC, NM, 128], BF16, kind="ExternalInput")
    wk2_d = nc.dram_tensor("wk2", [128, KC, NM, 128], BF16, kind="ExternalInput")
    wr2_d = nc.dram_tensor("wr2", [128, KC, NM, 128], BF16, kind="ExternalInput")
    b2m_d = nc.dram_tensor("b2m", [16, 128], BF16, kind="ExternalInput")
    wms_d = nc.dram_tensor("wms", [128, KC, 2], BF16, kind="ExternalInput")
    bms_d = nc.dram_tensor("bms", [1, 2], F32, kind="ExternalInput")
    dl16_d = nc.dram_tensor("dl16", [16, NM * BSH], BF16, kind="ExternalInput")

    mu_d = nc.dram_tensor("mu", [BSH, T], F32, kind="ExternalOutput")
    sg_d = nc.dram_tensor("sigma", [BSH, T], F32, kind="ExternalOutput")
    dbg_d = nc.dram_tensor("dbg", [128, T, BSH], F32, kind="ExternalOutput") \
        if upto != "all" else None

    _build_body(nc, upto, locals())
    nc.compile()
    _NC_CACHE[upto] = nc
    return nc


def _build_body(nc, upto, env):
    from contextlib import ExitStack
    idx0_d = env["idx0_d"]; idx1_d = env["idx1_d"]; xcr_d = env["xcr_d"]
    e0t_d = env["e0t_d"]; e1t_d = env["e1t_d"]; w1e_d = env["w1e_d"]
    wr1_d = env["wr1_d"]; wk2_d = env["wk2_d"]; wr2_d = env["wr2_d"]
    b2m_d = env["b2m_d"]; wms_d = env["wms_d"]; bms_d = env["bms_d"]
    dl16_d = env["dl16_d"]
    mu_d = env["mu_d"]; sg_d = env["sg_d"]; dbg_d = env["dbg_d"]

    with tile.TileContext(nc) as tc, ExitStack() as top:
        singles = top.enter_context(tc.tile_pool(name="singles", bufs=1))

        # ---------------- weights / constants to SBUF ----------------
        # gather inputs (idx/xcb) are DMA'd first — see phase 1 — so the
        # HWDGE queue isn't clogged by the 52KB/partition of weights
        w1e = singles.tile([64, NM, 128], BF16)
        wr1 = singles.tile([128, KC, NM, 128], BF16)
        wk2 = singles.tile([128, KC, NM, 128], BF16)
        wr2 = singles.tile([128, KC, NM, 128], BF16)
        b2m = singles.tile([16, 128], BF16)
        nc.sync.dma_start(out=b2m[:], in_=b2m_d[:])
        wms = singles.tile([128, KC, 2], BF16)
        nc.sync.dma_start(out=wms[:], in_=wms_d[:])
        bms = singles.tile([1, 2], F32)
        nc.sync.dma_start(out=bms[:], in_=bms_d[:])
        # bmu/bsig broadcast to 32 partitions for the epilogue
        bmu32 = singles.tile([BSH, 1], F32)
        nc.sync.dma_start(
            out=bmu32[:],
            in_=bass.AP(tensor=bms_d[:].tensor, offset=0, ap=[[0, BSH], [1, 1]]))
        bsg32 = singles.tile([BSH, 1], F32)
        nc.sync.dma_start(
            out=bsg32[:],
            in_=bass.AP(tensor=bms_d[:].tensor, offset=1, ap=[[0, BSH], [1, 1]]))

        ident_f32 = singles.tile([128, 128], F32)
        make_identity(nc, ident_f32[:])

        # delta16[k, (m, j)] = 1 if k == m else 0 — b2-inject rhs
        delta16 = singles.tile([16, NM, BSH], BF16)
        nc.sync.dma_start(out=delta16[:], in_=dl16_d[:])

        # h^T rings, cell states (bf16), head staging
        h1h = singles.tile([128, R1, KC, BSH], BF16)
        h2h = singles.tile([128, R2, KC, BSH], BF16)
        c1 = singles.tile([128, KC, BSH], BF16)
        c2 = singles.tile([128, KC, BSH], BF16)
        stage = singles.tile([BSH, T, 2], F32)

        # ---------------- phase 1: gather + x^T build ----------------
        gp = top.enter_context(tc.tile_pool(name="gather", bufs=1))
        idx0_sb = gp.tile([128, MT], I32)
        nc.sync.dma_start(out=idx0_sb[:], in_=idx0_d[:])
        idx1_sb = gp.tile([128, MT], I32)
        nc.sync.dma_start(out=idx1_sb[:], in_=idx1_d[:])
        # assembled rows: [p, m, 64] = [e0 | e1 | xc | ones(+pad)]
        asm = gp.tile([128, MT, 64], F32)
        nc.vector.memset(asm[:], 1.0)
        xcb = gp.tile([128, MT, F], F32)
        nc.sync.dma_start(out=xcb[:], in_=xcr_d[:])
        nc.vector.tensor_copy(asm[:, :, E0 + E1:DIN], xcb[:])
        # weights after the gather inputs; first-use order
        nc.sync.dma_start(out=w1e[:], in_=w1e_d[:])
        nc.sync.dma_start(out=wr1[:], in_=wr1_d[:])
        nc.sync.dma_start(out=wk2[:], in_=wk2_d[:])
        nc.sync.dma_start(out=wr2[:], in_=wr2_d[:])
        for m in range(MT):
            nc.gpsimd.indirect_dma_start(
                out=asm[:, m, 0:E0], out_offset=None, in_=e0t_d[:],
                in_offset=bass.IndirectOffsetOnAxis(
                    ap=idx0_sb[:, m:m + 1], axis=0))
            nc.gpsimd.indirect_dma_start(
                out=asm[:, m, E0:E0 + E1], out_offset=None, in_=e1t_d[:],
                in_offset=bass.IndirectOffsetOnAxis(
                    ap=idx1_sb[:, m:m + 1], axis=0))

        xT = top.enter_context(tc.tile_pool(name="xtp", bufs=1)).tile([64, R], BF16)
        ptr = top.enter_context(tc.tile_pool(name="gtr", bufs=1, space="PSUM"))

        def xT_tile(m):
            ps = ptr.tile([64, 128], F32, tag="tr")
            nc.tensor.transpose(ps[:], asm[:, m, :], ident_f32[:])
            nc.vector.tensor_copy(xT[:, 128 * m:128 * (m + 1)], ps[:])

        xT_tile(0)
        xT_tile(1)
        xT_done = 2

        if upto == "xT":
            for m in range(2, MT):
                xT_tile(m)
            with tc.tile_pool(name="dbgp", bufs=1) as dp:
                dbg_sb = dp.tile([64, R], F32)
                nc.vector.tensor_copy(dbg_sb[:], xT[:])
                dv = bass.AP(tensor=dbg_d[:].tensor, offset=0,
                             ap=[[T * BSH, 64], [1, R]])
                nc.sync.dma_start(out=dv, in_=dbg_sb[:])
            return
        dbg_sb = None
        if upto in ("h1", "h2"):
            dbg_sb = singles.tile([128, T, BSH], F32)

        # ---------------- main interleaved scan ----------------
        ew = top.enter_context(tc.tile_pool(name="ew", bufs=2))
        psg1 = top.enter_context(tc.tile_pool(name="psg1", bufs=2, space="PSUM"))
        psg2 = top.enter_context(tc.tile_pool(name="psg2", bufs=3, space="PSUM"))
        psh = top.enter_context(tc.tile_pool(name="psh", bufs=2, space="PSUM"))

        pg1 = {}
        pg2 = {}
        psH = [None]

        def b2inj(s):
            # starts the L2 psum group for step s (zeroes the bank)
            pg = psg2.tile([128, NM, BSH], F32, tag="pg2", name="pg2")
            pg2[s] = pg
            nc.tensor.matmul(pg[:], b2m[:], delta16[:],
                             start=True, stop=False, skip_group_check=True)

        def xzb1(t):
            # input projection + b1 for step t (starts the psum group)
            pg = psg1.tile([128, NM, BSH], F32, tag="pg1")
            pg1[t] = pg
            # start=True zeroes the whole 2KB zero region (= this bank):
            # only the first matmul into the bank may set it
            for m in range(NM):
                nc.tensor.matmul(pg[:, m, :], w1e[0:57, m, :],
                                 xT[0:57, t * BSH:(t + 1) * BSH],
                                 start=(m == 0),
                                 stop=(t == 0 and m == NM - 1),
                                 skip_group_check=True)

        def wr_mm(pg, w, hring, rslot, t0):
            for m in range(NM):
                for c in range(KC):
                    nc.tensor.matmul(pg[:, m, :], w[:, c, m, :],
                                     hring[:, rslot, c, :],
                                     start=False,
                                     stop=(t0 and m == NM - 1 and c == KC - 1),
                                     skip_group_check=True)

        def tail(t, pg, cst, hring, rslot):
            # gates^T [128, 16, 32] -> h^T ring slot; slots [i | f | o | g]
            sig = ew.tile([128, 12, BSH], BF16, tag="sig")
            nc.scalar.activation(sig[:], pg[:, 0:12, :], A.Sigmoid)
            tng = ew.tile([128, KC, BSH], BF16, tag="tng")
            nc.scalar.activation(tng[:], pg[:, 12:16, :], A.Tanh)
            ig = ew.tile([128, KC, BSH], BF16, tag="ig")
            if t > 0:
                fc = ew.tile([128, KC, BSH], BF16, tag="fc")
                nc.vector.tensor_mul(fc[:], sig[:, KC:2 * KC, :], cst[:])
                nc.vector.tensor_mul(ig[:], sig[:, 0:KC, :], tng[:])
                nc.vector.tensor_add(cst[:], fc[:], ig[:])
            else:
                nc.vector.tensor_mul(ig[:], sig[:, 0:KC, :], tng[:])
                nc.vector.tensor_copy(cst[:], ig[:])
            tnc = ew.tile([128, KC, BSH], BF16, tag="tnc")
            nc.scalar.activation(tnc[:], cst[:], A.Tanh)
            nc.vector.tensor_mul(hring[:, rslot, :, :],
                                 sig[:, 2 * KC:3 * KC, :], tnc[:])

        def head(s):
            # mu/sigma for step s: out [32, 2] <- sum_c h2^T_c(stat) @ wms_c
            if s % 16 == 0:
                psH[0] = psh.tile([BSH, 16, 2], F32, tag="psH", name="psH")
            for c in range(KC):
                nc.tensor.matmul(psH[0][:, s % 16, :], h2h[:, s % R2, c, :],
                                 wms[:, c, :],
                                 start=(c == 0 and s % 16 == 0),
                                 stop=(c == KC - 1 and s % 16 == 15),
                                 skip_group_check=True)
            if s % 16 == 15:
                nc.vector.tensor_copy(stage[:, s - 15:s + 1, :], psH[0][:])

        # macro step t, PE order: [Wk2(t-2) | Wr2(t-2) | head(t-3) |
        # Wr1(t) | xzb1(t+1) | b2(t-1)]; ACT order: L1 tail(t) then L2
        # tail(t-2). L2 lags L1 by 2 so its gates are complete at macro
        # start and both tails' ACT work fits inside one PE period.
        xzb1(0)
        for t in range(T + 3):
            s = t - 2
            l2mm = None
            if 0 <= s < T:
                pg = pg2.pop(s)
                wr_mm(pg, wk2, h1h, s % R1, s == 0)
                if s > 0:
                    wr_mm(pg, wr2, h2h, (s - 1) % R2, True)
                l2mm = pg
            if 0 <= t - 3 < T:
                head(t - 3)
            if t < T:
                if t > 0:
                    wr_mm(pg1[t], wr1, h1h, (t - 1) % R1, True)
                tail(t, pg1.pop(t), c1, h1h, t % R1)
            if l2mm is not None:
                tail(s, l2mm, c2, h2h, s % R2)
            if t + 1 < T:
                xzb1(t + 1)
            if t < T:
                b2inj(t)
            while xT_done * 4 < t + 10 and xT_done < MT:
                xT_tile(xT_done)
                xT_done += 1
            if upto == "h1" and t < T:
                nc.vector.tensor_copy(dbg_sb[:, t, :], h1h[:, t % R1, 0, :])
            if upto == "h2" and 0 <= s < T:
                nc.vector.tensor_copy(dbg_sb[:, s, :], h2h[:, s % R2, 0, :])

        # ---------------- epilogue: bias, softplus, DMA out ----------------
        ep = top.enter_context(tc.tile_pool(name="ep", bufs=1))
        mu_sb = ep.tile([BSH, T], F32)
        nc.vector.tensor_scalar_add(mu_sb[:], stage[:, :, 0], bmu32[:])
        sg_e = ep.tile([BSH, T], F32)
        nc.scalar.activation(sg_e[:], stage[:, :, 1], A.Exp, bias=bsg32[:])
        sg_sb = ep.tile([BSH, T], F32)
        nc.scalar.activation(sg_sb[:], sg_e[:], A.Ln, bias=1.0)
        nc.sync.dma_start(out=mu_d[:], in_=mu_sb[:])
        nc.sync.dma_start(out=sg_d[:], in_=sg_sb[:])
        if upto in ("h1", "h2"):
            nc.sync.dma_start(out=dbg_d[:], in_=dbg_sb[:])

    return nc


def _marshal(inputs):
    """Host-side shard/layout marshalling (no compute beyond dtype cast/pad)."""
    bf = ml_dtypes.bfloat16
    xc = np.ascontiguousarray(np.asarray(inputs["x_cont"], np.float32))
    cat0 = np.asarray(inputs["cat0"]).astype(np.int32)
    cat1 = np.asarray(inputs["cat1"]).astype(np.int32)
    emb0 = np.asarray(inputs["emb0"], np.float32)
    emb1 = np.asarray(inputs["emb1"], np.float32)
    Wk1 = np.asarray(inputs["Wk1"], np.float32)
    Wr1 = np.asarray(inputs["Wr1"], np.float32)
    b1 = np.asarray(inputs["b1"], np.float32)
    Wk2 = np.asarray(inputs["Wk2"], np.float32)
    Wr2 = np.asarray(inputs["Wr2"], np.float32)
    b2 = np.asarray(inputs["b2"], np.float32)
    Wmu = np.asarray(inputs["Wmu"], np.float32)
    bmu = np.asarray(inputs["bmu"], np.float32)
    Wsig = np.asarray(inputs["Wsig"], np.float32)
    bsig = np.asarray(inputs["bsig"], np.float32)

    P = _colperm()

    # xT partition order: 0-31 emb0 dims, 32-47 emb1 dims, 48-55 x_cont, 56 ones
    wk1_rows = np.concatenate([Wk1[F:F + E0], Wk1[F + E0:DIN], Wk1[0:F],
                               b1.reshape(1, G4)], axis=0)      # [57, G4]
    w1e = np.zeros((64, NM, 128), bf)
    w1e[0:57] = wk1_rows[:, P].reshape(57, NM, 128).astype(bf)

    def kperm(W):  # [512, G4] -> [128(k), KC, NM, 128]
        Wp = W[:, P].reshape(KC, 128, NM, 128)
        return np.ascontiguousarray(Wp.transpose(1, 0, 2, 3)).astype(bf)

    wr1 = kperm(Wr1)
    wk2 = kperm(Wk2)
    wr2 = kperm(Wr2)
    b2m = np.ascontiguousarray(b2[P].reshape(16, 128)).astype(bf)
    wms = np.zeros((128, KC, 2), bf)
    for c in range(KC):
        wms[:, c, 0] = Wmu[c * 128:(c + 1) * 128, 0].astype(bf)
        wms[:, c, 1] = Wsig[c * 128:(c + 1) * 128, 0].astype(bf)
    bms = np.array([[float(bmu.reshape(-1)[0]), float(bsig.reshape(-1)[0])]],
                   np.float32)
    dl16 = np.kron(np.eye(16, dtype=np.float32),
                   np.ones((1, BSH), np.float32)).astype(bf)  # [16, 512]

    def wrap_idx(cat):  # [BSH, T] -> (t,b) rows -> [128, MT] int32
        lin = np.ascontiguousarray(cat.T).reshape(-1)       # (t, b) order
        return np.ascontiguousarray(lin.reshape(MT, 128).T.astype(np.int32))

    in_maps = []
    for cidx in range(NC_N):
        sl = slice(cidx * BSH, (cidx + 1) * BSH)
        xcs = xc[sl]                                        # [32, 192, 8]
        rows = xcs.transpose(1, 0, 2).reshape(R, F)         # (t,b) rows
        xcr = np.ascontiguousarray(
            rows.reshape(MT, 128, F).transpose(1, 0, 2).astype(np.float32))
        in_maps.append({
            "xcr": xcr,
            "idx0": wrap_idx(cat0[sl]),
            "idx1": wrap_idx(cat1[sl]),
            "e0tab": emb0, "e1tab": emb1,
            "w1e": w1e, "wr1": wr1, "wk2": wk2, "wr2": wr2,
            "b2m": b2m, "wms": wms, "bms": bms, "dl16": dl16,
        })
    return in_maps


_RUN_KWARGS = {}   # test harness may set e.g. {"trace": True} for profiling
_LAST_RESULT = []


def kernel(**inputs):
    from concourse.bass_utils import run_bass_kernel_spmd
    in_maps = _marshal(inputs)
    nc = build_nc()
    res = run_bass_kernel_spmd(nc, in_maps, core_ids=list(range(NC_N)),
                               **_RUN_KWARGS)
    _LAST_RESULT.clear()
    _LAST_RESULT.append(res)
    mu = np.concatenate([r["mu"] for r in res.results], axis=0)      # [256, 192]
    sg = np.concatenate([r["sigma"] for r in res.results], axis=0)
    return (mu.reshape(B, T, 1).astype(np.float32),
            sg.reshape(B, T, 1).astype(np.float32))


# revision 32
# speedup vs baseline: 1.2149x; 1.0073x over previous
"""DeepAR (2-layer LSTM, H=512) Trainium2 Bass kernel, 8-core data-parallel.

Model (see reference): x = concat(x_cont, emb0[cat0], emb1[cat1]) [B,T,56]
  -> LSTM(512) -> LSTM(512) -> mu = h@Wmu+bmu ; sigma = softplus(h@Wsig+bsig)

Sharding: batch B=256 split across 8 cores (32 rows each); params replicated.

Per-core device program — transposed-gates formulation. All recurrent
matmuls put the WEIGHTS in the stationary operand and stream h^T, so each
matmul's moving dim is just the 32-row batch:

  gates^T [2048, 32] = sum_c Wr[c]^T-tiles @ h^T_c  (+ [Wk;b] @ [x^T;1])

The PE streams 16 Mtiles x (1 xz + 4 Wr) x 32 rows for L1 and
16 x (4 Wk2 + 4 Wr2) x 32 (+512-row b2 inject) for L2 — ~7.2K rows/step
vs ~21.5K for the batch-major formulation, and h^T is produced directly
by the elementwise tail (no per-step PE transposes).

  - gate tile order [i | f | o | g]: one fused Sigmoid over 12 tiles +
    one Tanh over 4; cell update and h = o*tanh(c) on DVE in bf16
  - both layers interleaved in one scan (L2 runs 1 step behind L1);
    PE order per macro step: Wr1[t], b2/Wk2/Wr2[t-1], head[t-1],
    xzb1[t+1] — the L1 ACT/DVE tail hides under the L2 matmuls
  - head: lhsT = h2^T chunk (stationary), rhs = Wms [128, 2] — 8 rows
    per step, accumulated 16 steps per PSUM bank, staged batch-major
  - embeddings gathered up-front on the Pool queue (96 single-index
    indirect DMAs; multi-index indirect DMA corrupts SBUF on HW), with
    the x^T PE transposes pipelined 2 tiles ahead of the scan
  - all Exp/Ln (softplus) deferred to one epilogue pass -> no ACT
    table swaps inside the scan
"""

import numpy as np
import ml_dtypes

import concourse.bass as bass
import concourse.mybir as mybir
import concourse.tile as tile
from concourse import bacc
from concourse.masks import make_identity

F32 = mybir.dt.float32
BF16 = mybir.dt.bfloat16
I32 = mybir.dt.int32

B, T, F = 256, 192, 8
CARD0, CARD1 = 1000, 100
E0, E1 = 32, 16
H = 512
DIN = F + E0 + E1          # 56
G4 = 4 * H                 # 2048
NC_N = 8                   # cores
BSH = B // NC_N            # 32 batch rows per core
R = T * BSH                # 6144 (t,b)-ordered rows per core
KC = H // 128              # 4 recurrent K-chunks
NM = G4 // 128             # 16 gate-column Mtiles
MT = R // 128              # 48 x^T column tiles
R1 = 4                     # h1^T ring depth
R2 = 8                     # h2^T ring depth
A = mybir.ActivationFunctionType

# gate slot order [i | f | o | g] (i: slots 0-3, f: 4-7, o: 8-11, g: 12-15)
_GATE_BASE = [0, H, 3 * H, 2 * H]   # orig col base per slot-group, z=[i|f|g|o]


def _colperm():
    """P[m*128+p] = original G4 column of (Mtile m, partition p)."""
    P = np.empty(G4, np.int64)
    for m in range(NM):
        base = _GATE_BASE[m // 4]
        chunk = m % 4
        P[m * 128:(m + 1) * 128] = base + chunk * 128 + np.arange(128)
    return P


_NC_CACHE = {}


def build_nc(upto="all"):
    if upto in _NC_CACHE:
        return _NC_CACHE[upto]
    nc = bacc.Bacc("TRN2", num_devices=NC_N)

    # ---------------- DRAM I/O ----------------
    idx0_d = nc.dram_tensor("idx0", [128, MT], I32, kind="ExternalInput")
    idx1_d = nc.dram_tensor("idx1", [128, MT], I32, kind="ExternalInput")
    e0t_d = nc.dram_tensor("e0tab", [CARD0, E0], F32, kind="ExternalInput")
    e1t_d = nc.dram_tensor("e1tab", [CARD1, E1], F32, kind="ExternalInput")
    xcr_d = nc.dram_tensor("xcr", [128, MT, F], F32, kind="ExternalInput")
    w1e_d = nc.dram_tensor("w1e", [64, NM, 128], BF16, kind="ExternalInput")
    wr1_d = nc.dram_tensor("wr1", [128, KC, NM, 128], BF16, kind="ExternalInput")
    wk2_d = nc.dram_tensor("wk2", [128, KC, NM, 128], BF16, kind="ExternalInput")
    wr2_d = nc.dram_tensor("wr2", [128, KC, NM, 128], BF16, kind="ExternalInput")
    b2m_d = nc.dram_tensor("b2m", [16, 128], BF16, kind="ExternalInput")
    wms_d = nc.dram_tensor("wms", [128, KC, 2], BF16, kind="ExternalInput")
    bms_d = nc.dram_tensor("bms", [1, 2], F32, kind="ExternalInput")
    dl16_d = nc.dram_tensor("dl16", [16, NM * BSH], BF16, kind="ExternalInput")

    mu_d = nc.dram_tensor("mu", [BSH, T], F32, kind="ExternalOutput")
    sg_d = nc.dram_tensor("sigma", [BSH, T], F32, kind="ExternalOutput")
    dbg_d = nc.dram_tensor("dbg", [128, T, BSH], F32, kind="ExternalOutput") \
        if upto != "all" else None

    _build_body(nc, upto, locals())
    nc.compile()
    _NC_CACHE[upto] = nc
    return nc


def _build_body(nc, upto, env):
    from contextlib import ExitStack
    idx0_d = env["idx0_d"]; idx1_d = env["idx1_d"]; xcr_d = env["xcr_d"]
    e0t_d = env["e0t_d"]; e1t_d = env["e1t_d"]; w1e_d = env["w1e_d"]
    wr1_d = env["wr1_d"]; wk2_d = env["wk2_d"]; wr2_d = env["wr2_d"]
    b2m_d = env["b2m_d"]; wms_d = env["wms_d"]; bms_d = env["bms_d"]
    dl16_d = env["dl16_d"]
    mu_d = env["mu_d"]; sg_d = env["sg_d"]; dbg_d = env["dbg_d"]

    with tile.TileContext(nc) as tc, ExitStack() as top:
        singles = top.enter_context(tc.tile_pool(name="singles", bufs=1))

        # ---------------- weights / constants to SBUF ----------------
        # gather inputs (idx/xcb) are DMA'd first — see phase 1 — so the
        # HWDGE queue isn't clogged by the 52KB/partition of weights
        w1e = singles.tile([64, NM, 128], BF16)
        wr1 = singles.tile([128, KC, NM, 128], BF16)
        wk2 = singles.tile([128, KC, NM, 128], BF16)
        wr2 = singles.tile([128, KC, NM, 128], BF16)
        b2m = singles.tile([16, 128], BF16)
        nc.sync.dma_start(out=b2m[:], in_=b2m_d[:])
        wms = singles.tile([128, KC, 2], BF16)
        nc.sync.dma_start(out=wms[:], in_=wms_d[:])
        bms = singles.tile([1, 2], F32)
        nc.sync.dma_start(out=bms[:], in_=bms_d[:])
        # bmu/bsig broadcast to 32 partitions for the epilogue
        bmu32 = singles.tile([BSH, 1], F32)
        nc.sync.dma_start(
            out=bmu32[:],
            in_=bass.AP(tensor=bms_d[:].tensor, offset=0, ap=[[0, BSH], [1, 1]]))
        bsg32 = singles.tile([BSH, 1], F32)
        nc.sync.dma_start(
            out=bsg32[:],
            in_=bass.AP(tensor=bms_d[:].tensor, offset=1, ap=[[0, BSH], [1, 1]]))

        ident_f32 = singles.tile([128, 128], F32)
        make_identity(nc, ident_f32[:])

        # delta16[k, (m, j)] = 1 if k == m else 0 — b2-inject rhs
        delta16 = singles.tile([16, NM, BSH], BF16)
        nc.sync.dma_start(out=delta16[:], in_=dl16_d[:])

        # h^T rings, cell states (bf16), head staging
        h1h = singles.tile([128, R1, KC, BSH], BF16)
        h2h = singles.tile([128, R2, KC, BSH], BF16)
        c1 = singles.tile([128, KC, BSH], BF16)
        c2 = singles.tile([128, KC, BSH], BF16)
        stage = singles.tile([BSH, T, 2], F32)

        # ---------------- phase 1: gather + x^T build ----------------
        gp = top.enter_context(tc.tile_pool(name="gather", bufs=1))
        idx0_sb = gp.tile([128, MT], I32)
        nc.sync.dma_start(out=idx0_sb[:], in_=idx0_d[:])
        idx1_sb = gp.tile([128, MT], I32)
        nc.sync.dma_start(out=idx1_sb[:], in_=idx1_d[:])
        # assembled rows: [p, m, 64] = [e0 | e1 | xc | ones(+pad)]
        asm = gp.tile([128, MT, 64], F32)
        nc.vector.memset(asm[:], 1.0)
        xcb = gp.tile([128, MT, F], F32)
        nc.sync.dma_start(out=xcb[:], in_=xcr_d[:])
        nc.vector.tensor_copy(asm[:, :, E0 + E1:DIN], xcb[:])
        # weights after the gather inputs; first-use order
        nc.sync.dma_start(out=w1e[:], in_=w1e_d[:])
        nc.sync.dma_start(out=wr1[:], in_=wr1_d[:])
        nc.sync.dma_start(out=wk2[:], in_=wk2_d[:])
        nc.sync.dma_start(out=wr2[:], in_=wr2_d[:])
        for m in range(MT):
            nc.gpsimd.indirect_dma_start(
                out=asm[:, m, 0:E0], out_offset=None, in_=e0t_d[:],
                in_offset=bass.IndirectOffsetOnAxis(
                    ap=idx0_sb[:, m:m + 1], axis=0))
            nc.gpsimd.indirect_dma_start(
                out=asm[:, m, E0:E0 + E1], out_offset=None, in_=e1t_d[:],
                in_offset=bass.IndirectOffsetOnAxis(
                    ap=idx1_sb[:, m:m + 1], axis=0))

        xT = top.enter_context(tc.tile_pool(name="xtp", bufs=1)).tile([64, R], BF16)
        ptr = top.enter_context(tc.tile_pool(name="gtr", bufs=1, space="PSUM"))

        def xT_tile(m):
            ps = ptr.tile([64, 128], F32, tag="tr")
            nc.tensor.transpose(ps[:], asm[:, m, :], ident_f32[:])
            nc.vector.tensor_copy(xT[:, 128 * m:128 * (m + 1)], ps[:])

        xT_tile(0)
        xT_tile(1)
        xT_done = 2

        if upto == "xT":
            for m in range(2, MT):
                xT_tile(m)
            with tc.tile_pool(name="dbgp", bufs=1) as dp:
                dbg_sb = dp.tile([64, R], F32)
                nc.vector.tensor_copy(dbg_sb[:], xT[:])
                dv = bass.AP(tensor=dbg_d[:].tensor, offset=0,
                             ap=[[T * BSH, 64], [1, R]])
                nc.sync.dma_start(out=dv, in_=dbg_sb[:])
            return
        dbg_sb = None
        if upto in ("h1", "h2"):
            dbg_sb = singles.tile([128, T, BSH], F32)

        # ---------------- main interleaved scan ----------------
        ew = top.enter_context(tc.tile_pool(name="ew", bufs=2))
        psg1 = top.enter_context(tc.tile_pool(name="psg1", bufs=2, space="PSUM"))
        psg2 = top.enter_context(tc.tile_pool(name="psg2", bufs=3, space="PSUM"))
        psh = top.enter_context(tc.tile_pool(name="psh", bufs=2, space="PSUM"))

        pg1 = {}
        pg2 = {}
        psH = [None]

        def b2inj(s):
            # starts the L2 psum group for step s (zeroes the bank)
            pg = psg2.tile([128, NM, BSH], F32, tag="pg2", name="pg2")
            pg2[s] = pg
            nc.tensor.matmul(pg[:], b2m[:], delta16[:],
                             start=True, stop=False, skip_group_check=True)

        def xzb1(t):
            # input projection + b1 for step t (starts the psum group)
            pg = psg1.tile([128, NM, BSH], F32, tag="pg1")
            pg1[t] = pg
            # start=True zeroes the whole 2KB zero region (= this bank):
            # only the first matmul into the bank may set it
            for m in range(NM):
                nc.tensor.matmul(pg[:, m, :], w1e[0:57, m, :],
                                 xT[0:57, t * BSH:(t + 1) * BSH],
                                 start=(m == 0),
                                 stop=(t == 0 and m == NM - 1),
                                 skip_group_check=True)

        def wr_mm(pg, w, hring, rslot, t0):
            for m in range(NM):
                for c in range(KC):
                    nc.tensor.matmul(pg[:, m, :], w[:, c, m, :],
                                     hring[:, rslot, c, :],
                                     start=False,
                                     stop=(t0 and m == NM - 1 and c == KC - 1),
                                     skip_group_check=True)

        def tail_a(t, pg, cst, lyr):
            # gate nonlinearities + cell update; slots [i | f | o | g]
            sig = ew.tile([128, 12, BSH], BF16, tag=f"sig{lyr}", name="sig")
            nc.scalar.activation(sig[:], pg[:, 0:12, :], A.Sigmoid)
            tng = ew.tile([128, KC, BSH], BF16, tag=f"tng{lyr}", name="tng")
            nc.scalar.activation(tng[:], pg[:, 12:16, :], A.Tanh)
            ig = ew.tile([128, KC, BSH], BF16, tag=f"ig{lyr}", name="ig")
            if t > 0:
                fc = ew.tile([128, KC, BSH], BF16, tag=f"fc{lyr}", name="fc")
                nc.vector.tensor_mul(fc[:], sig[:, KC:2 * KC, :], cst[:])
                nc.vector.tensor_mul(ig[:], sig[:, 0:KC, :], tng[:])
                nc.vector.tensor_add(cst[:], fc[:], ig[:])
            else:
                nc.vector.tensor_mul(ig[:], sig[:, 0:KC, :], tng[:])
                nc.vector.tensor_copy(cst[:], ig[:])
            return sig

        def tail_b(sig, cst, hring, rslot, lyr):
            # h^T = sigmoid(o) * tanh(c) -> ring slot
            tnc = ew.tile([128, KC, BSH], BF16, tag=f"tnc{lyr}", name="tnc")
            nc.scalar.activation(tnc[:], cst[:], A.Tanh)
            nc.vector.tensor_mul(hring[:, rslot, :, :],
                                 sig[:, 2 * KC:3 * KC, :], tnc[:])

        def head(s):
            # mu/sigma for step s: out [32, 2] <- sum_c h2^T_c(stat) @ wms_c
            if s % 16 == 0:
                psH[0] = psh.tile([BSH, 16, 2], F32, tag="psH", name="psH")
            for c in range(KC):
                nc.tensor.matmul(psH[0][:, s % 16, :], h2h[:, s % R2, c, :],
                                 wms[:, c, :],
                                 start=(c == 0 and s % 16 == 0),
                                 stop=(c == KC - 1 and s % 16 == 15),
                                 skip_group_check=True)
            if s % 16 == 15:
                nc.vector.tensor_copy(stage[:, s - 15:s + 1, :], psH[0][:])

        # macro step t. L2 lags L1 by 2 steps so every dependency arrives
        # a full macro early. PE order:
        #   [Wk2(t-2) | Wr1(t) | xzb1(t+1) | b2(t) | Wr2(t-2) | head(t-3)]
        # ACT order: [sig1(t), tng1(t), tnc1(t), sig2(t-2), tng2, tnc2] —
        # L2's sigmoid fills the bubble while the L1 DVE c-chain runs.
        xzb1(0)
        for t in range(T + 3):
            s = t - 2
            if 0 <= s < T:
                wr_mm(pg2[s], wk2, h1h, s % R1, s == 0)
            if 0 < t < T:
                wr_mm(pg1[t], wr1, h1h, (t - 1) % R1, True)
            if t + 1 < T:
                xzb1(t + 1)
            if t < T:
                b2inj(t)
            if t < T:
                sig1t = tail_a(t, pg1.pop(t), c1, 1)
                tail_b(sig1t, c1, h1h, t % R1, 1)
            if 0 <= s < T:
                if s > 0:
                    wr_mm(pg2[s], wr2, h2h, (s - 1) % R2, True)
                if 0 <= t - 3 < T:
                    head(t - 3)
                sig2t = tail_a(s, pg2.pop(s), c2, 2)
                tail_b(sig2t, c2, h2h, s % R2, 2)
            elif 0 <= t - 3 < T:
                head(t - 3)
            while xT_done * 4 < t + 10 and xT_done < MT:
                xT_tile(xT_done)
                xT_done += 1
            if upto == "h1" and t < T:
                nc.vector.tensor_copy(dbg_sb[:, t, :], h1h[:, t % R1, 0, :])
            if upto == "h2" and 0 <= s < T:
                nc.vector.tensor_copy(dbg_sb[:, s, :], h2h[:, s % R2, 0, :])

        # ---------------- epilogue: bias, softplus, DMA out ----------------
        ep = top.enter_context(tc.tile_pool(name="ep", bufs=1))
        mu_sb = ep.tile([BSH, T], F32)
        nc.vector.tensor_scalar_add(mu_sb[:], stage[:, :, 0], bmu32[:])
        sg_e = ep.tile([BSH, T], F32)
        nc.scalar.activation(sg_e[:], stage[:, :, 1], A.Exp, bias=bsg32[:])
        sg_sb = ep.tile([BSH, T], F32)
        nc.scalar.activation(sg_sb[:], sg_e[:], A.Ln, bias=1.0)
        nc.sync.dma_start(out=mu_d[:], in_=mu_sb[:])
        nc.sync.dma_start(out=sg_d[:], in_=sg_sb[:])
        if upto in ("h1", "h2"):
            nc.sync.dma_start(out=dbg_d[:], in_=dbg_sb[:])

    return nc


def _marshal(inputs):
    """Host-side shard/layout marshalling (no compute beyond dtype cast/pad)."""
    bf = ml_dtypes.bfloat16
    xc = np.ascontiguousarray(np.asarray(inputs["x_cont"], np.float32))
    cat0 = np.asarray(inputs["cat0"]).astype(np.int32)
    cat1 = np.asarray(inputs["cat1"]).astype(np.int32)
    emb0 = np.asarray(inputs["emb0"], np.float32)
    emb1 = np.asarray(inputs["emb1"], np.float32)
    Wk1 = np.asarray(inputs["Wk1"], np.float32)
    Wr1 = np.asarray(inputs["Wr1"], np.float32)
    b1 = np.asarray(inputs["b1"], np.float32)
    Wk2 = np.asarray(inputs["Wk2"], np.float32)
    Wr2 = np.asarray(inputs["Wr2"], np.float32)
    b2 = np.asarray(inputs["b2"], np.float32)
    Wmu = np.asarray(inputs["Wmu"], np.float32)
    bmu = np.asarray(inputs["bmu"], np.float32)
    Wsig = np.asarray(inputs["Wsig"], np.float32)
    bsig = np.asarray(inputs["bsig"], np.float32)

    P = _colperm()

    # xT partition order: 0-31 emb0 dims, 32-47 emb1 dims, 48-55 x_cont, 56 ones
    wk1_rows = np.concatenate([Wk1[F:F + E0], Wk1[F + E0:DIN], Wk1[0:F],
                               b1.reshape(1, G4)], axis=0)      # [57, G4]
    w1e = np.zeros((64, NM, 128), bf)
    w1e[0:57] = wk1_rows[:, P].reshape(57, NM, 128).astype(bf)

    def kperm(W):  # [512, G4] -> [128(k), KC, NM, 128]
        Wp = W[:, P].reshape(KC, 128, NM, 128)
        return np.ascontiguousarray(Wp.transpose(1, 0, 2, 3)).astype(bf)

    wr1 = kperm(Wr1)
    wk2 = kperm(Wk2)
    wr2 = kperm(Wr2)
    b2m = np.ascontiguousarray(b2[P].reshape(16, 128)).astype(bf)
    wms = np.zeros((128, KC, 2), bf)
    for c in range(KC):
        wms[:, c, 0] = Wmu[c * 128:(c + 1) * 128, 0].astype(bf)
        wms[:, c, 1] = Wsig[c * 128:(c + 1) * 128, 0].astype(bf)
    bms = np.array([[float(bmu.reshape(-1)[0]), float(bsig.reshape(-1)[0])]],
                   np.float32)
    dl16 = np.kron(np.eye(16, dtype=np.float32),
                   np.ones((1, BSH), np.float32)).astype(bf)  # [16, 512]

    def wrap_idx(cat):  # [BSH, T] -> (t,b) rows -> [128, MT] int32
        lin = np.ascontiguousarray(cat.T).reshape(-1)       # (t, b) order
        return np.ascontiguousarray(lin.reshape(MT, 128).T.astype(np.int32))

    in_maps = []
    for cidx in range(NC_N):
        sl = slice(cidx * BSH, (cidx + 1) * BSH)
        xcs = xc[sl]                                        # [32, 192, 8]
        rows = xcs.transpose(1, 0, 2).reshape(R, F)         # (t,b) rows
        xcr = np.ascontiguousarray(
            rows.reshape(MT, 128, F).transpose(1, 0, 2).astype(np.float32))
        in_maps.append({
            "xcr": xcr,
            "idx0": wrap_idx(cat0[sl]),
            "idx1": wrap_idx(cat1[sl]),
            "e0tab": emb0, "e1tab": emb1,
            "w1e": w1e, "wr1": wr1, "wk2": wk2, "wr2": wr2,
            "b2m": b2m, "wms": wms, "bms": bms, "dl16": dl16,
        })
    return in_maps


_RUN_KWARGS = {}   # test harness may set e.g. {"trace": True} for profiling
_LAST_RESULT = []


def kernel(**inputs):
    from concourse.bass_utils import run_bass_kernel_spmd
    in_maps = _marshal(inputs)
    nc = build_nc()
    res = run_bass_kernel_spmd(nc, in_maps, core_ids=list(range(NC_N)),
                               **_RUN_KWARGS)
    _LAST_RESULT.clear()
    _LAST_RESULT.append(res)
    mu = np.concatenate([r["mu"] for r in res.results], axis=0)      # [256, 192]
    sg = np.concatenate([r["sigma"] for r in res.results], axis=0)
    return (mu.reshape(B, T, 1).astype(np.float32),
            sg.reshape(B, T, 1).astype(np.float32))


# revision 33
# speedup vs baseline: 1.3009x; 1.0708x over previous
"""DeepAR (2-layer LSTM, H=512) Trainium2 Bass kernel, 8-core data-parallel.

Model (see reference): x = concat(x_cont, emb0[cat0], emb1[cat1]) [B,T,56]
  -> LSTM(512) -> LSTM(512) -> mu = h@Wmu+bmu ; sigma = softplus(h@Wsig+bsig)

Sharding: batch B=256 split across 8 cores (32 rows each); params replicated.

Per-core device program — transposed-gates formulation. All recurrent
matmuls put the WEIGHTS in the stationary operand and stream h^T, so each
matmul's moving dim is just the 32-row batch:

  gates^T [2048, 32] = sum_c Wr[c]^T-tiles @ h^T_c  (+ [Wk;b] @ [x^T;1])

The PE streams 16 Mtiles x (1 xz + 4 Wr) x 32 rows for L1 and
16 x (4 Wk2 + 4 Wr2) x 32 (+512-row b2 inject) for L2 — ~7.2K rows/step
vs ~21.5K for the batch-major formulation, and h^T is produced directly
by the elementwise tail (no per-step PE transposes).

  - gate tile order [i | f | o | g]: one fused Sigmoid over 12 tiles +
    one Tanh over 4; cell update and h = o*tanh(c) on DVE in bf16
  - both layers interleaved in one scan (L2 runs 1 step behind L1);
    PE order per macro step: Wr1[t], b2/Wk2/Wr2[t-1], head[t-1],
    xzb1[t+1] — the L1 ACT/DVE tail hides under the L2 matmuls
  - head: lhsT = h2^T chunk (stationary), rhs = Wms [128, 2] — 8 rows
    per step, accumulated 16 steps per PSUM bank, staged batch-major
  - embeddings gathered up-front on the Pool queue (96 single-index
    indirect DMAs; multi-index indirect DMA corrupts SBUF on HW), with
    the x^T PE transposes pipelined 2 tiles ahead of the scan
  - all Exp/Ln (softplus) deferred to one epilogue pass -> no ACT
    table swaps inside the scan
"""

import numpy as np
import ml_dtypes

import concourse.bass as bass
import concourse.mybir as mybir
import concourse.tile as tile
from concourse import bacc
from concourse.masks import make_identity

F32 = mybir.dt.float32
BF16 = mybir.dt.bfloat16
FP8 = mybir.dt.float8e4
I32 = mybir.dt.int32
SW = 64.0   # fp8 weight scale; un-done by ACT scale=1/SW on the gate reads

B, T, F = 256, 192, 8
CARD0, CARD1 = 1000, 100
E0, E1 = 32, 16
H = 512
DIN = F + E0 + E1          # 56
G4 = 4 * H                 # 2048
NC_N = 8                   # cores
BSH = B // NC_N            # 32 batch rows per core
R = T * BSH                # 6144 (t,b)-ordered rows per core
KC = H // 128              # 4 recurrent K-chunks
NM = G4 // 128             # 16 gate-column Mtiles
MT = R // 128              # 48 x^T column tiles
R1 = 4                     # h1^T ring depth
R2 = 8                     # h2^T ring depth
A = mybir.ActivationFunctionType

# gate slot order [i | f | o | g] (i: slots 0-3, f: 4-7, o: 8-11, g: 12-15)
_GATE_BASE = [0, H, 3 * H, 2 * H]   # orig col base per slot-group, z=[i|f|g|o]


def _colperm():
    """P[m*128+p] = original G4 column of (Mtile m, partition p)."""
    P = np.empty(G4, np.int64)
    for m in range(NM):
        base = _GATE_BASE[m // 4]
        chunk = m % 4
        P[m * 128:(m + 1) * 128] = base + chunk * 128 + np.arange(128)
    return P


_NC_CACHE = {}


def build_nc(upto="all"):
    if upto in _NC_CACHE:
        return _NC_CACHE[upto]
    nc = bacc.Bacc("TRN2", num_devices=NC_N)

    # ---------------- DRAM I/O ----------------
    idx0_d = nc.dram_tensor("idx0", [128, MT], I32, kind="ExternalInput")
    idx1_d = nc.dram_tensor("idx1", [128, MT], I32, kind="ExternalInput")
    e0t_d = nc.dram_tensor("e0tab", [CARD0, E0], F32, kind="ExternalInput")
    e1t_d = nc.dram_tensor("e1tab", [CARD1, E1], F32, kind="ExternalInput")
    xcr_d = nc.dram_tensor("xcr", [128, MT, F], F32, kind="ExternalInput")
    w1e_d = nc.dram_tensor("w1e", [64, NM, 128], BF16, kind="ExternalInput")
    wr1_d = nc.dram_tensor("wr1", [128, 2, 2, NM, 128], FP8, kind="ExternalInput")
    wk2_d = nc.dram_tensor("wk2", [128, KC, NM, 128], BF16, kind="ExternalInput")
    wr2_d = nc.dram_tensor("wr2", [128, 2, 2, NM, 128], FP8, kind="ExternalInput")
    b2m_d = nc.dram_tensor("b2m", [16, 128], BF16, kind="ExternalInput")
    wms_d = nc.dram_tensor("wms", [128, KC, 2], BF16, kind="ExternalInput")
    bms_d = nc.dram_tensor("bms", [1, 2], F32, kind="ExternalInput")
    dl16_d = nc.dram_tensor("dl16", [16, NM * BSH], BF16, kind="ExternalInput")

    mu_d = nc.dram_tensor("mu", [BSH, T], F32, kind="ExternalOutput")
    sg_d = nc.dram_tensor("sigma", [BSH, T], F32, kind="ExternalOutput")
    dbg_d = nc.dram_tensor("dbg", [128, T, BSH], F32, kind="ExternalOutput") \
        if upto != "all" else None

    _build_body(nc, upto, locals())
    nc.compile()
    _NC_CACHE[upto] = nc
    return nc


def _build_body(nc, upto, env):
    from contextlib import ExitStack
    idx0_d = env["idx0_d"]; idx1_d = env["idx1_d"]; xcr_d = env["xcr_d"]
    e0t_d = env["e0t_d"]; e1t_d = env["e1t_d"]; w1e_d = env["w1e_d"]
    wr1_d = env["wr1_d"]; wk2_d = env["wk2_d"]; wr2_d = env["wr2_d"]
    b2m_d = env["b2m_d"]; wms_d = env["wms_d"]; bms_d = env["bms_d"]
    dl16_d = env["dl16_d"]
    mu_d = env["mu_d"]; sg_d = env["sg_d"]; dbg_d = env["dbg_d"]

    with tile.TileContext(nc) as tc, ExitStack() as top:
        singles = top.enter_context(tc.tile_pool(name="singles", bufs=1))

        # ---------------- weights / constants to SBUF ----------------
        # gather inputs (idx/xcb) are DMA'd first — see phase 1 — so the
        # HWDGE queue isn't clogged by the 52KB/partition of weights
        w1e = singles.tile([64, NM, 128], BF16)
        wr1 = singles.tile([128, 2, 2, NM, 128], FP8)
        wk2 = singles.tile([128, KC, NM, 128], BF16)
        wr2 = singles.tile([128, 2, 2, NM, 128], FP8)
        b2m = singles.tile([16, 128], BF16)
        nc.sync.dma_start(out=b2m[:], in_=b2m_d[:])
        wms = singles.tile([128, KC, 2], BF16)
        nc.sync.dma_start(out=wms[:], in_=wms_d[:])
        bms = singles.tile([1, 2], F32)
        nc.sync.dma_start(out=bms[:], in_=bms_d[:])
        # bmu/bsig broadcast to 32 partitions for the epilogue
        bmu32 = singles.tile([BSH, 1], F32)
        nc.sync.dma_start(
            out=bmu32[:],
            in_=bass.AP(tensor=bms_d[:].tensor, offset=0, ap=[[0, BSH], [1, 1]]))
        bsg32 = singles.tile([BSH, 1], F32)
        nc.sync.dma_start(
            out=bsg32[:],
            in_=bass.AP(tensor=bms_d[:].tensor, offset=1, ap=[[0, BSH], [1, 1]]))

        ident_f32 = singles.tile([128, 128], F32)
        make_identity(nc, ident_f32[:])

        # delta16[k, (m, j)] = 1 if k == m else 0 — b2-inject rhs
        delta16 = singles.tile([16, NM, BSH], BF16)
        nc.sync.dma_start(out=delta16[:], in_=dl16_d[:])

        # h^T rings, cell states (bf16), head staging
        h1h = singles.tile([128, R1, KC, BSH], FP8)
        h2h = singles.tile([128, R2, KC, BSH], FP8)
        h2hb = singles.tile([128, R2, KC, BSH], BF16)
        c1 = singles.tile([128, KC, BSH], BF16)
        c2 = singles.tile([128, KC, BSH], BF16)
        stage = singles.tile([BSH, T, 2], F32)

        # ---------------- phase 1: gather + x^T build ----------------
        gp = top.enter_context(tc.tile_pool(name="gather", bufs=1))
        idx0_sb = gp.tile([128, MT], I32)
        nc.sync.dma_start(out=idx0_sb[:], in_=idx0_d[:])
        idx1_sb = gp.tile([128, MT], I32)
        nc.sync.dma_start(out=idx1_sb[:], in_=idx1_d[:])
        # assembled rows: [p, m, 64] = [e0 | e1 | xc | ones(+pad)]
        asm = gp.tile([128, MT, 64], F32)
        nc.vector.memset(asm[:], 1.0)
        xcb = gp.tile([128, MT, F], F32)
        nc.sync.dma_start(out=xcb[:], in_=xcr_d[:])
        nc.vector.tensor_copy(asm[:, :, E0 + E1:DIN], xcb[:])
        # weights after the gather inputs; first-use order
        nc.sync.dma_start(out=w1e[:], in_=w1e_d[:])
        nc.sync.dma_start(out=wr1[:], in_=wr1_d[:])
        nc.sync.dma_start(out=wk2[:], in_=wk2_d[:])
        nc.sync.dma_start(out=wr2[:], in_=wr2_d[:])
        for m in range(MT):
            nc.gpsimd.indirect_dma_start(
                out=asm[:, m, 0:E0], out_offset=None, in_=e0t_d[:],
                in_offset=bass.IndirectOffsetOnAxis(
                    ap=idx0_sb[:, m:m + 1], axis=0))
            nc.gpsimd.indirect_dma_start(
                out=asm[:, m, E0:E0 + E1], out_offset=None, in_=e1t_d[:],
                in_offset=bass.IndirectOffsetOnAxis(
                    ap=idx1_sb[:, m:m + 1], axis=0))

        xT = top.enter_context(tc.tile_pool(name="xtp", bufs=1)).tile([64, R], BF16)
        ptr = top.enter_context(tc.tile_pool(name="gtr", bufs=1, space="PSUM"))

        def xT_tile(m):
            ps = ptr.tile([64, 128], F32, tag="tr")
            nc.tensor.transpose(ps[:], asm[:, m, :], ident_f32[:])
            nc.vector.tensor_copy(xT[:, 128 * m:128 * (m + 1)], ps[:])

        xT_tile(0)
        xT_tile(1)
        xT_done = 2

        if upto == "xT":
            for m in range(2, MT):
                xT_tile(m)
            with tc.tile_pool(name="dbgp", bufs=1) as dp:
                dbg_sb = dp.tile([64, R], F32)
                nc.vector.tensor_copy(dbg_sb[:], xT[:])
                dv = bass.AP(tensor=dbg_d[:].tensor, offset=0,
                             ap=[[T * BSH, 64], [1, R]])
                nc.sync.dma_start(out=dv, in_=dbg_sb[:])
            return
        dbg_sb = None
        if upto in ("h1", "h2"):
            dbg_sb = singles.tile([128, T, BSH], F32)

        # ---------------- main interleaved scan ----------------
        ew = top.enter_context(tc.tile_pool(name="ew", bufs=2))
        psg1 = top.enter_context(tc.tile_pool(name="psg1", bufs=2, space="PSUM"))
        psg2 = top.enter_context(tc.tile_pool(name="psg2", bufs=3, space="PSUM"))
        psh = top.enter_context(tc.tile_pool(name="psh", bufs=2, space="PSUM"))

        pg1 = {}
        pg2 = {}
        psH = [None]

        def b2inj(s):
            # starts the L2 psum group for step s (zeroes the bank)
            pg = psg2.tile([128, NM, BSH], F32, tag="pg2", name="pg2")
            pg2[s] = pg
            nc.tensor.matmul(pg[:], b2m[:], delta16[:],
                             start=True, stop=False, skip_group_check=True)

        def xzb1(t):
            # input projection + b1 for step t (starts the psum group)
            pg = psg1.tile([128, NM, BSH], F32, tag="pg1")
            pg1[t] = pg
            # start=True zeroes the whole 2KB zero region (= this bank):
            # only the first matmul into the bank may set it
            for m in range(NM):
                nc.tensor.matmul(pg[:, m, :], w1e[0:57, m, :],
                                 xT[0:57, t * BSH:(t + 1) * BSH],
                                 start=(m == 0),
                                 stop=(t == 0 and m == NM - 1),
                                 skip_group_check=True)

        def wr_mm(pg, w, hring, rslot, t0):
            for m in range(NM):
                for c in range(KC):
                    nc.tensor.matmul(pg[:, m, :], w[:, c, m, :],
                                     hring[:, rslot, c, :],
                                     start=False,
                                     stop=(t0 and m == NM - 1 and c == KC - 1),
                                     skip_group_check=True)

        def wr_mm_f8(pg, w, hring, rslot, t0):
            # fp8 DoubleRow: K=256 per matmul (pairs on AP dim 1)
            for m in range(NM):
                for c2 in range(2):
                    nc.tensor.matmul(pg[:, m, :], w[:, c2, :, m, :],
                                     hring[:, rslot, 2 * c2:2 * c2 + 2, :],
                                     start=False,
                                     stop=(t0 and m == NM - 1 and c2 == 1),
                                     perf_mode=mybir.MatmulPerfMode.DoubleRow,
                                     skip_group_check=True)

        def tail_a(t, pg, cst, lyr):
            # gate nonlinearities + cell update; slots [i | f | o | g]
            sig = ew.tile([128, 12, BSH], BF16, tag=f"sig{lyr}", name="sig")
            nc.scalar.activation(sig[:], pg[:, 0:12, :], A.Sigmoid, scale=1.0 / SW)
            tng = ew.tile([128, KC, BSH], BF16, tag=f"tng{lyr}", name="tng")
            nc.scalar.activation(tng[:], pg[:, 12:16, :], A.Tanh, scale=1.0 / SW)
            ig = ew.tile([128, KC, BSH], BF16, tag=f"ig{lyr}", name="ig")
            if t > 0:
                fc = ew.tile([128, KC, BSH], BF16, tag=f"fc{lyr}", name="fc")
                nc.vector.tensor_mul(fc[:], sig[:, KC:2 * KC, :], cst[:])
                nc.vector.tensor_mul(ig[:], sig[:, 0:KC, :], tng[:])
                nc.vector.tensor_add(cst[:], fc[:], ig[:])
            else:
                nc.vector.tensor_mul(ig[:], sig[:, 0:KC, :], tng[:])
                nc.vector.tensor_copy(cst[:], ig[:])
            return sig

        def tail_b(sig, cst, hring, rslot, lyr, hringb=None):
            # h^T = sigmoid(o) * tanh(c) -> fp8 ring (recurrent consumers);
            # bf16 copy of h2 for the head off the critical cycle
            tnc = ew.tile([128, KC, BSH], BF16, tag=f"tnc{lyr}", name="tnc")
            nc.scalar.activation(tnc[:], cst[:], A.Tanh)
            nc.vector.tensor_mul(hring[:, rslot, :, :],
                                 sig[:, 2 * KC:3 * KC, :], tnc[:])
            if hringb is not None:
                nc.vector.tensor_mul(hringb[:, rslot, :, :],
                                     sig[:, 2 * KC:3 * KC, :], tnc[:])

        def head(s):
            # mu/sigma for step s: out [32, 2] <- sum_c h2^T_c(stat) @ wms_c
            if s % 16 == 0:
                psH[0] = psh.tile([BSH, 16, 2], F32, tag="psH", name="psH")
            for c in range(KC):
                nc.tensor.matmul(psH[0][:, s % 16, :], h2hb[:, s % R2, c, :],
                                 wms[:, c, :],
                                 start=(c == 0 and s % 16 == 0),
                                 stop=(c == KC - 1 and s % 16 == 15),
                                 skip_group_check=True)
            if s % 16 == 15:
                nc.vector.tensor_copy(stage[:, s - 15:s + 1, :], psH[0][:])

        # macro step t. L2 lags L1 by 2 steps so every dependency arrives
        # a full macro early. PE order:
        #   [Wk2(t-2) | Wr1(t) | xzb1(t+1) | b2(t) | Wr2(t-2) | head(t-3)]
        # ACT order: [sig1(t), tng1(t), tnc1(t), sig2(t-2), tng2, tnc2] —
        # L2's sigmoid fills the bubble while the L1 DVE c-chain runs.
        xzb1(0)
        for t in range(T + 3):
            s = t - 2
            if 0 <= s < T:
                wr_mm(pg2[s], wk2, h1h, s % R1, s == 0)
            if 0 < t < T:
                wr_mm_f8(pg1[t], wr1, h1h, (t - 1) % R1, True)
            if t + 1 < T:
                xzb1(t + 1)
            if t < T:
                b2inj(t)
            if t < T:
                sig1t = tail_a(t, pg1.pop(t), c1, 1)
                tail_b(sig1t, c1, h1h, t % R1, 1)
            if 0 <= s < T:
                if s > 0:
                    wr_mm_f8(pg2[s], wr2, h2h, (s - 1) % R2, True)
                if 0 <= t - 3 < T:
                    head(t - 3)
                sig2t = tail_a(s, pg2.pop(s), c2, 2)
                tail_b(sig2t, c2, h2h, s % R2, 2, hringb=h2hb)
            elif 0 <= t - 3 < T:
                head(t - 3)
            while xT_done * 4 < t + 10 and xT_done < MT:
                xT_tile(xT_done)
                xT_done += 1
            if upto == "h1" and t < T:
                nc.vector.tensor_copy(dbg_sb[:, t, :], h1h[:, t % R1, 0, :])
            if upto == "h2" and 0 <= s < T:
                nc.vector.tensor_copy(dbg_sb[:, s, :], h2h[:, s % R2, 0, :])

        # ---------------- epilogue: bias, softplus, DMA out ----------------
        ep = top.enter_context(tc.tile_pool(name="ep", bufs=1))
        mu_sb = ep.tile([BSH, T], F32)
        nc.vector.tensor_scalar_add(mu_sb[:], stage[:, :, 0], bmu32[:])
        sg_e = ep.tile([BSH, T], F32)
        nc.scalar.activation(sg_e[:], stage[:, :, 1], A.Exp, bias=bsg32[:])
        sg_sb = ep.tile([BSH, T], F32)
        nc.scalar.activation(sg_sb[:], sg_e[:], A.Ln, bias=1.0)
        nc.sync.dma_start(out=mu_d[:], in_=mu_sb[:])
        nc.sync.dma_start(out=sg_d[:], in_=sg_sb[:])
        if upto in ("h1", "h2"):
            nc.sync.dma_start(out=dbg_d[:], in_=dbg_sb[:])

    return nc


def _marshal(inputs):
    """Host-side shard/layout marshalling (no compute beyond dtype cast/pad)."""
    bf = ml_dtypes.bfloat16
    xc = np.ascontiguousarray(np.asarray(inputs["x_cont"], np.float32))
    cat0 = np.asarray(inputs["cat0"]).astype(np.int32)
    cat1 = np.asarray(inputs["cat1"]).astype(np.int32)
    emb0 = np.asarray(inputs["emb0"], np.float32)
    emb1 = np.asarray(inputs["emb1"], np.float32)
    Wk1 = np.asarray(inputs["Wk1"], np.float32)
    Wr1 = np.asarray(inputs["Wr1"], np.float32)
    b1 = np.asarray(inputs["b1"], np.float32)
    Wk2 = np.asarray(inputs["Wk2"], np.float32)
    Wr2 = np.asarray(inputs["Wr2"], np.float32)
    b2 = np.asarray(inputs["b2"], np.float32)
    Wmu = np.asarray(inputs["Wmu"], np.float32)
    bmu = np.asarray(inputs["bmu"], np.float32)
    Wsig = np.asarray(inputs["Wsig"], np.float32)
    bsig = np.asarray(inputs["bsig"], np.float32)

    P = _colperm()
    f8 = ml_dtypes.float8_e4m3fn

    # xT partition order: 0-31 emb0 dims, 32-47 emb1 dims, 48-55 x_cont, 56 ones
    # input-projection weights and biases carry the x{SW} fp8 scale
    wk1_rows = np.concatenate([Wk1[F:F + E0], Wk1[F + E0:DIN], Wk1[0:F],
                               b1.reshape(1, G4)], axis=0) * SW  # [57, G4]
    w1e = np.zeros((64, NM, 128), bf)
    w1e[0:57] = wk1_rows[:, P].reshape(57, NM, 128).astype(bf)

    def kperm(W, scale=1.0):  # [512, G4] -> [128(k), KC, NM, 128]
        Wp = (W[:, P] * scale).reshape(KC, 128, NM, 128)
        return np.ascontiguousarray(Wp.transpose(1, 0, 2, 3)).astype(bf)

    def kperm8(W):  # [512, G4] -> [128(k), 2(c2), 2(pair), NM, 128] fp8
        Wp = (W[:, P] * SW).reshape(2, 2, 128, NM, 128)
        return np.ascontiguousarray(Wp.transpose(2, 0, 1, 3, 4)).astype(f8)

    wr1 = kperm8(Wr1)
    wk2 = kperm(Wk2, SW)
    wr2 = kperm8(Wr2)
    b2m = np.ascontiguousarray(b2[P].reshape(16, 128) * SW).astype(bf)
    wms = np.zeros((128, KC, 2), bf)
    for c in range(KC):
        wms[:, c, 0] = Wmu[c * 128:(c + 1) * 128, 0].astype(bf)
        wms[:, c, 1] = Wsig[c * 128:(c + 1) * 128, 0].astype(bf)
    bms = np.array([[float(bmu.reshape(-1)[0]), float(bsig.reshape(-1)[0])]],
                   np.float32)
    dl16 = np.kron(np.eye(16, dtype=np.float32),
                   np.ones((1, BSH), np.float32)).astype(bf)  # [16, 512]

    def wrap_idx(cat):  # [BSH, T] -> (t,b) rows -> [128, MT] int32
        lin = np.ascontiguousarray(cat.T).reshape(-1)       # (t, b) order
        return np.ascontiguousarray(lin.reshape(MT, 128).T.astype(np.int32))

    in_maps = []
    for cidx in range(NC_N):
        sl = slice(cidx * BSH, (cidx + 1) * BSH)
        xcs = xc[sl]                                        # [32, 192, 8]
        rows = xcs.transpose(1, 0, 2).reshape(R, F)         # (t,b) rows
        xcr = np.ascontiguousarray(
            rows.reshape(MT, 128, F).transpose(1, 0, 2).astype(np.float32))
        in_maps.append({
            "xcr": xcr,
            "idx0": wrap_idx(cat0[sl]),
            "idx1": wrap_idx(cat1[sl]),
            "e0tab": emb0, "e1tab": emb1,
            "w1e": w1e, "wr1": wr1, "wk2": wk2, "wr2": wr2,
            "b2m": b2m, "wms": wms, "bms": bms, "dl16": dl16,
        })
    return in_maps


_RUN_KWARGS = {}   # test harness may set e.g. {"trace": True} for profiling
_LAST_RESULT = []


def kernel(**inputs):
    from concourse.bass_utils import run_bass_kernel_spmd
    in_maps = _marshal(inputs)
    nc = build_nc()
    res = run_bass_kernel_spmd(nc, in_maps, core_ids=list(range(NC_N)),
                               **_RUN_KWARGS)
    _LAST_RESULT.clear()
    _LAST_RESULT.append(res)
    mu = np.concatenate([r["mu"] for r in res.results], axis=0)      # [256, 192]
    sg = np.concatenate([r["sigma"] for r in res.results], axis=0)
    return (mu.reshape(B, T, 1).astype(np.float32),
            sg.reshape(B, T, 1).astype(np.float32))
